# revision 33
# baseline (speedup 1.0000x reference)
"""Trainium2 Bass kernel for nn_BestHits: out = bh * bh.T where
bh = blockwise-softmax(mask_diag(similarities) / TAU) over 256-wide column groups.

Strategy: out is symmetric, so only the 136 upper-incl-diagonal 512x512
block-pairs are computed (17 per core on 8 cores); out[J,I] = out[I,J].T is
mirrored on the host.

fp16 end-to-end to halve HBM traffic (memory-bound problem): the host
subtracts the per-(row, 256-group) max before quantizing to fp16 (softmax is
shift-invariant, and the shift makes fp16 rounding error negligible: the
dominant softmax terms sit near 0 where fp16 absolute error is tiny).
Outputs are stored fp16 and widened to f32 on the host.

Per pair (I, J) the device sees A = Y[I, J]-block and W-input = Y[J, I].T
(transposed on host during staging, so the reciprocal product is purely
elementwise on device -- no PE transpose):
  out[I,J][r, c] = za[r, c]/sa[r, g(c)] * W[r, c]/sB[g(r), c]
with za = exp(10*A), W = exp(10*B.T),
  sa[r, g]  = row-group sums of za        (DVE fp16 tree-adds + short reduce),
  sB[g, c]  = partition-group sums of W   (PE ones-matmuls, PSUM-accumulated,
              output already broadcast across all 128 partitions).
A-sums come from the DVE fp16 tree for 9 slots and from ACT accum_out
(8-way split exp) for the other 8 -- the split that balances the two
loaded engines. All sums land in spare columns of the B-sum PSUM tile so
ONE reciprocal_approx_fast covers both normalizations; the fp16 downcast
of the combined normalizer runs on ACT so the V-multiply hits the DVE
fp16 2x path. The slot loop is software-pipelined: front half of k+1 is
emitted before the back half of k, and each slot's 8-stt final product is
delayed a further iteration so DVE always holds ready work while the
newer slot's recip chain crosses PE/ACT. Stock RECIPROCAL measures
~3.3us/instr, DVE has no divide ALU op, and TensorScalarPtr never reaches
its fast modes on HW -- hence this exact op selection. Measured: DVE
~109us busy, ACT ~106, DMA ~92, PE ~34; 136.7us exec vs the 174us f32
baseline (same-device measurement of the staged 185.6us baseline).
"""
import sys

import numpy as np

sys.path.insert(0, "/opt/trn_rl_repo")

from contextlib import ExitStack

import concourse.bass as bass  # noqa: F401  (registers AP machinery)
import concourse.tile as tile
from concourse import bacc, mybir
from concourse.bass_utils import run_bass_kernel_spmd

N = 8192          # full matrix side
B = 512           # block side
NB = N // B       # 16 blocks per side
P = 128           # SBUF partitions
T = B // P        # 4 row-subtiles per block
GRP = 256         # softmax group width
NG = B // GRP     # 2 groups per block side
TAU = 0.1
NSLOTS = 17       # block-pairs per core
NCORES = 8
YCLIP = -512.0    # lower clip for max-subtracted values (exp(10*y) == 0)

F16 = mybir.dt.float16
F32 = mybir.dt.float32
AF = mybir.ActivationFunctionType
OP = mybir.AluOpType

def core_pairs() -> list[list[tuple[int, int]]]:
    """136 upper-triangle block pairs distributed 17-per-core (2 diagonal
    pairs last per core; the device treats all slots uniformly)."""
    diag = [(i, i) for i in range(NB)]
    off = [(i, j) for i in range(NB) for j in range(i + 1, NB)]
    cps: list[list[tuple[int, int]]] = [[] for _ in range(NCORES)]
    for idx, p in enumerate(off):
        cps[idx % NCORES].append(p)
    for idx, p in enumerate(diag):
        cps[idx % NCORES].append(p)
    return cps


CORE_PAIRS = core_pairs()


def build():
    """Build + compile the (single-program, 8-core SPMD) Bass kernel."""
    nc = bacc.Bacc(
        "TRN2",
        target_bir_lowering=False,
        debug=False,
        enable_asserts=True,
        num_devices=NCORES,
    )
    a = nc.dram_tensor("a", [NSLOTS, P, T, B], F16, kind="ExternalInput").ap()
    w = nc.dram_tensor("w", [NSLOTS, P, T, B], F16, kind="ExternalInput").ap()
    o = nc.dram_tensor("o", [NSLOTS, P, T, B], F16, kind="ExternalOutput").ap()

    with tile.TileContext(nc) as tc, ExitStack() as ctx:
        const_pool = ctx.enter_context(tc.tile_pool(name="const", bufs=1))
        ones = const_pool.tile([P, P], F16)
        nc.vector.memset(ones[:], 1.0)

        a_pool = ctx.enter_context(tc.tile_pool(name="a_sb", bufs=4))
        w_pool = ctx.enter_context(tc.tile_pool(name="w_sb", bufs=4))
        za_pool = ctx.enter_context(tc.tile_pool(name="za", bufs=4))
        wz_pool = ctx.enter_context(tc.tile_pool(name="wz", bufs=4))
        v_pool = ctx.enter_context(tc.tile_pool(name="vv", bufs=3))
        o_pool = ctx.enter_context(tc.tile_pool(name="o_sb", bufs=4))
        st_pool = ctx.enter_context(tc.tile_pool(name="st", bufs=8))
        rb_pool = ctx.enter_context(tc.tile_pool(name="rb", bufs=2))
        rb16_pool = ctx.enter_context(tc.tile_pool(name="rb16", bufs=3))
        ps_pool = ctx.enter_context(tc.tile_pool(name="ps", bufs=2, space="PSUM"))

        # Slots whose A-side sums come from ACT accum_out (8-way split exp,
        # +~2.0us ACT) instead of the DVE tree+reduce (-~1.7us DVE): engine
        # rebalance, ACT ~88us -> ~105 and DVE ~120 -> ~107.
        ACCUM_SLOTS = frozenset(range(1, NSLOTS, 2))

        def stage_a1(k):
            """Loads of slot k (+ A-side exp for tree slots)."""
            a_sb = a_pool.tile([P, T, B], F16)
            nc.sync.dma_start(a_sb[:], a[k])
            w_sb = w_pool.tile([P, T, B], F16)
            nc.sync.dma_start(w_sb[:], w[k])
            za = za_pool.tile([P, T, B], F16)
            if k not in ACCUM_SLOTS:
                nc.scalar.activation(za[:], a_sb[:], AF.Exp, scale=1.0 / TAU)
            return za, a_sb, w_sb

        def stage_a2(k, za, a_sb, w_sb):
            """W-side exp, PE B-sums, A-sums (tree or ACT accum) of slot k."""
            wz = wz_pool.tile([P, T, B], F16)
            nc.scalar.activation(wz[:], w_sb[:], AF.Exp, scale=1.0 / TAU)

            # B-side group sums on PE: sb_ps[:, g, c] = sum over the 256
            # B-columns of group g (= partitions of wz t-subtiles 2g, 2g+1),
            # broadcast to all 128 output partitions by the all-ones lhsT.
            # The tile carries T spare columns per group that the A-side
            # reduce fills below, so ONE reciprocal_approx_fast covers both
            # normalizations (the approx-recip has ~0.7us fixed cost). Group
            # stride padded to 1024 f32 (2 PSUM banks) so each matmul output
            # sits inside bank boundaries.
            sb_ps = ps_pool.tile([P, NG, 2 * B], F32)
            for t in range(T):
                nc.tensor.matmul(
                    sb_ps[:, t // 2, 0:B], ones[:], wz[:, t, :],
                    start=(t % 2 == 0), stop=(t % 2 == 1),
                )

            sb_flat = sb_ps[:].rearrange("p g c -> p (g c)")
            if k in ACCUM_SLOTS:
                # A-exp as 8 per-group activations whose accum_out drops each
                # group sum straight into the spare PSUM columns.
                for t in range(T):
                    for g in range(NG):
                        cs = slice(g * GRP, (g + 1) * GRP)
                        col = g * (2 * B) + B + t
                        nc.scalar.activation(
                            za[:, t, cs], a_sb[:, t, cs], AF.Exp,
                            scale=1.0 / TAU,
                            accum_out=sb_flat[:, col:col + 1],
                        )
            else:
                # A-side group sums: fp16 tree-add halvings at DVE 2x, then
                # a short f32 tensor_reduce (free-dim reduces are DVE-only,
                # and plain tensor_reduce has no fp16 fast mode).
                za_g = za[:].rearrange("p t b -> p (t b)").rearrange(
                    "p (G s) -> p G s", s=GRP
                )
                s1 = st_pool.tile([P, T * NG, GRP // 2], F16, name="s1")
                nc.vector.tensor_add(
                    s1[:], za_g[:, :, 0:128], za_g[:, :, 128:256])
                s2 = st_pool.tile([P, T * NG, GRP // 4], F16, name="s2")
                nc.vector.tensor_add(s2[:], s1[:, :, 0:64], s1[:, :, 64:128])
                s3 = st_pool.tile([P, T * NG, GRP // 8], F16, name="s3")
                nc.vector.tensor_add(s3[:], s2[:, :, 0:32], s2[:, :, 32:64])
                # A-sums land t-major in the spare columns: [p, g, B + t].
                nc.vector.tensor_reduce(
                    sb_ps[:, :, B:B + T].rearrange("p g t -> p t g"),
                    s3[:], axis=mybir.AxisListType.X, op=OP.add,
                )
            return za, wz, sb_ps

        def stage_b1(k, sb_ps):
            """Combined reciprocal + fp16 downcast of slot k's normalizers.
            Emitted between slot k+1's two exps so the ACT-side downcast runs
            early instead of queueing behind both k+1 activations."""
            rbinv = rb_pool.tile([P, NG, B + T], F32)
            nc.vector.reciprocal_approx_fast(rbinv[:], sb_ps[:, :, 0:B + T])
            rb16 = rb16_pool.tile([P, NG, B + T], F16)
            nc.scalar.activation(rb16[:], rbinv[:], AF.Copy)
            return rb16

        def stage_b2(k, za, wz, rb16):
            """V-multiply of slot k."""
            v = v_pool.tile([P, T, B], F16)
            rb_b = rb16[:, :, 0:B].rearrange("p g (one c) -> p g one c", one=1) \
                .broadcast_to([P, NG, T // NG, B])
            nc.vector.tensor_mul(
                v[:].rearrange("p (g u) b -> p g u b", g=NG), wz[:].rearrange(
                    "p (g u) b -> p g u b", g=NG), rb_b,
            )
            return v

        def stage_b3(k, za, rb16, v):
            """Final product + store of slot k -- emitted one iteration late
            so these always-ready stt fill DVE stalls while the newer slot's
            recip chain crosses PE/ACT."""
            # out = (za * ra) * V: 8 scalar_tensor_tensor; the fp16 scalar
            # 1/sa[t, g] sits at rb16[:, g, B + t].
            o_sb = o_pool.tile([P, T, B], F16)
            for t in range(T):
                for g in range(NG):
                    cs = slice(g * GRP, (g + 1) * GRP)
                    nc.vector.scalar_tensor_tensor(
                        o_sb[:, t, cs], za[:, t, cs],
                        rb16[:, g, B + t:B + t + 1],
                        v[:, t, cs], op0=OP.mult, op1=OP.mult,
                    )
            # Stores ride the SWDGE (gpsimd) ring: they never queue ahead of
            # the sync-ring loads, and the Pool-engine dispatch cost is tiny.
            # Two half-tile stores let the first half leave while the second
            # half's stt work finishes, shortening the per-slot tail.
            nc.gpsimd.dma_start(o[k, :, 0:2], o_sb[:, 0:2, :])
            nc.gpsimd.dma_start(o[k, :, 2:4], o_sb[:, 2:4, :])

        # Software pipelining, one slot of lookahead. Per iteration the
        # emission order is: a1(k+1) [loads + za-exp], b1(k) [recip + rb16
        # downcast -- the ACT copy slips between k+1's exps], a2(k+1)
        # [wz-exp + PE sums + A-tree], b2(k) [V + product + store]. The DVE
        # exec queue then always holds ready tree work while slot k's recip
        # chain crosses PE/ACT, and the rb16 copy isn't stuck behind both
        # k+1 exps on the queue-depth-0 ACT engine.
        cur = stage_a2(0, *stage_a1(0))
        pend = None
        for k in range(NSLOTS):
            nxt = stage_a2(k + 1, *stage_a1(k + 1)) if k + 1 < NSLOTS else None
            rb16 = stage_b1(k, cur[2])
            v = stage_b2(k, cur[0], cur[1], rb16)
            if pend is not None:
                stage_b3(*pend)
            pend = (k, cur[0], rb16, v)
            cur = nxt
        stage_b3(*pend)

    nc.compile()
    return nc


_NC = None


def _get_nc():
    global _NC
    if _NC is None:
        _NC = build()
    return _NC


def _to_pmajor(blocks: np.ndarray) -> np.ndarray:
    # (n, 512, 512) row-major -> (n, 128, 4, 512): row r = t*P + p lands at
    # [p, t, :], so every SBUF partition's bytes are contiguous in DRAM.
    n = blocks.shape[0]
    return np.ascontiguousarray(
        blocks.reshape(n, T, P, B).transpose(0, 2, 1, 3)
    )


def _shifted_fp16(sims: np.ndarray) -> np.ndarray:
    """Y = sims - per-(row, 256-col-group) max, diagonal masked, clipped and
    cast to fp16. Softmax over any 256-aligned column group of Y matches the
    reference's (softmax shift invariance)."""
    y = np.array(sims, dtype=np.float32, copy=True)
    # Mask BEFORE the max: the group max must be over surviving entries,
    # else a dominant diagonal shifts the whole group into fp16 underflow
    # and the on-device group sum becomes 0 (-> inf/NaN).
    np.fill_diagonal(y, -np.inf)
    m = y.reshape(N, N // GRP, GRP).max(axis=-1, keepdims=True)
    y = (y.reshape(N, N // GRP, GRP) - m).reshape(N, N)
    np.clip(y, YCLIP, 0.0, out=y)
    return y.astype(np.float16)


def make_in_maps(sims: np.ndarray) -> list[dict[str, np.ndarray]]:
    yf = _shifted_fp16(sims)
    in_maps = []
    for c in range(NCORES):
        a_stack = np.empty((NSLOTS, B, B), np.float16)
        w_stack = np.empty((NSLOTS, B, B), np.float16)
        for k, (i, j) in enumerate(CORE_PAIRS[c]):
            a_stack[k] = yf[i * B:(i + 1) * B, j * B:(j + 1) * B]
            w_stack[k] = yf[j * B:(j + 1) * B, i * B:(i + 1) * B].T
        in_maps.append({"a": _to_pmajor(a_stack), "w": _to_pmajor(w_stack)})
    return in_maps


def assemble(results: list[dict[str, np.ndarray]]) -> np.ndarray:
    out = np.empty((N, N), np.float32)
    for c in range(NCORES):
        o_pm = results[c]["o"]  # (NSLOTS, P, T, B) fp16 partition-major
        o_stack = o_pm.transpose(0, 2, 1, 3).reshape(NSLOTS, B, B).astype(
            np.float32
        )
        for k, (i, j) in enumerate(CORE_PAIRS[c]):
            out[i * B:(i + 1) * B, j * B:(j + 1) * B] = o_stack[k]
            if i != j:
                out[j * B:(j + 1) * B, i * B:(i + 1) * B] = o_stack[k].T
    return out


def run_on_hw(sims: np.ndarray, **spmd_kwargs):
    """Run the kernel on the 8 NeuronCores. Returns (out, BassKernelResults).

    The device occasionally throws a transient NRT_EXEC_UNIT_UNRECOVERABLE
    and needs ~a minute to come back, so failed runs are retried."""
    import time

    nc = _get_nc()
    in_maps = make_in_maps(sims)
    last_exc = None
    for attempt in range(3):
        if attempt:
            time.sleep(75)
        try:
            res = run_bass_kernel_spmd(
                nc, in_maps, core_ids=list(range(NCORES)), **spmd_kwargs
            )
            return assemble(res.results), res
        except Exception as exc:  # noqa: BLE001 - device flake, retry
            last_exc = exc
    raise last_exc


def kernel(similarities: np.ndarray) -> np.ndarray:
    sims = np.ascontiguousarray(similarities, dtype=np.float32)
    assert sims.shape == (N, N)
    out, _ = run_on_hw(sims)
    return out


if __name__ == "__main__":
    rng = np.random.default_rng(0)
    sims = rng.standard_normal((N, N), dtype=np.float32)
    out = kernel(similarities=sims)
    print("out", out.shape, out.dtype, float(out.max()))


# revision 34
# speedup vs baseline: 1.0445x; 1.0445x over previous
"""Trainium2 Bass kernel for nn_BestHits: out = bh * bh.T where
bh = blockwise-softmax(mask_diag(similarities) / TAU) over 256-wide column groups.

Strategy: out is symmetric, so only the 136 upper-incl-diagonal 512x512
block-pairs are computed (17 per core on 8 cores); out[J,I] = out[I,J].T is
mirrored on the host.

fp16 end-to-end to halve HBM traffic (memory-bound problem): the host
subtracts the per-(row, 256-group) max before quantizing to fp16 (softmax is
shift-invariant, and the shift makes fp16 rounding error negligible: the
dominant softmax terms sit near 0 where fp16 absolute error is tiny).
Outputs are stored fp16 and widened to f32 on the host.

Per pair (I, J) the device sees A = Y[I, J]-block and W-input = Y[J, I].T
(transposed on host during staging, so the reciprocal product is purely
elementwise on device -- no PE transpose):
  out[I,J][r, c] = za[r, c]/sa[r, g(c)] * W[r, c]/sB[g(r), c]
with za = exp(10*A), W = exp(10*B.T),
  sa[r, g]  = row-group sums of za        (DVE fp16 tree-adds + short reduce),
  sB[g, c]  = partition-group sums of W   (PE ones-matmuls, PSUM-accumulated,
              output already broadcast across all 128 partitions).
A-sums come from the DVE fp16 tree for 9 slots and from ACT accum_out
(8-way split exp) for the other 8 -- the split that balances the two
loaded engines. All sums land in spare columns of the B-sum PSUM tile so
ONE reciprocal_approx_fast covers both normalizations; the fp16 downcast
of the combined normalizer runs on ACT so the V-multiply hits the DVE
fp16 2x path. The slot loop is software-pipelined: front half of k+1 is
emitted before the back half of k, and each slot's 8-stt final product is
delayed a further iteration so DVE always holds ready work while the
newer slot's recip chain crosses PE/ACT. Stock RECIPROCAL measures
~3.3us/instr, DVE has no divide ALU op, and TensorScalarPtr never reaches
its fast modes on HW -- hence this exact op selection. Measured: DVE
~109us busy, ACT ~106, DMA ~92, PE ~34; 136.7us exec vs the 174us f32
baseline (same-device measurement of the staged 185.6us baseline).
"""
import sys

import numpy as np

sys.path.insert(0, "/opt/trn_rl_repo")

from contextlib import ExitStack

import concourse.bass as bass  # noqa: F401  (registers AP machinery)
import concourse.tile as tile
from concourse import bacc, mybir
from concourse.bass_utils import run_bass_kernel_spmd

N = 8192          # full matrix side
B = 512           # block side
NB = N // B       # 16 blocks per side
P = 128           # SBUF partitions
T = B // P        # 4 row-subtiles per block
GRP = 256         # softmax group width
NG = B // GRP     # 2 groups per block side
TAU = 0.1
NSLOTS = 17       # block-pairs per core
NCORES = 8
YCLIP = -512.0    # lower clip for max-subtracted values (exp(10*y) == 0)

F16 = mybir.dt.float16
F32 = mybir.dt.float32
AF = mybir.ActivationFunctionType
OP = mybir.AluOpType

def core_pairs() -> list[list[tuple[int, int]]]:
    """136 upper-triangle block pairs distributed 17-per-core (2 diagonal
    pairs last per core; the device treats all slots uniformly)."""
    diag = [(i, i) for i in range(NB)]
    off = [(i, j) for i in range(NB) for j in range(i + 1, NB)]
    cps: list[list[tuple[int, int]]] = [[] for _ in range(NCORES)]
    for idx, p in enumerate(off):
        cps[idx % NCORES].append(p)
    for idx, p in enumerate(diag):
        cps[idx % NCORES].append(p)
    return cps


CORE_PAIRS = core_pairs()


def build():
    """Build + compile the (single-program, 8-core SPMD) Bass kernel."""
    nc = bacc.Bacc(
        "TRN2",
        target_bir_lowering=False,
        debug=False,
        enable_asserts=True,
        num_devices=NCORES,
    )
    a = nc.dram_tensor("a", [NSLOTS, P, T, B], F16, kind="ExternalInput").ap()
    w = nc.dram_tensor("w", [NSLOTS, P, T, B], F16, kind="ExternalInput").ap()
    o = nc.dram_tensor("o", [NSLOTS, P, T, B], F16, kind="ExternalOutput").ap()

    with tile.TileContext(nc) as tc, ExitStack() as ctx:
        const_pool = ctx.enter_context(tc.tile_pool(name="const", bufs=1))
        ones = const_pool.tile([P, P], F16)
        nc.vector.memset(ones[:], 1.0)

        a_pool = ctx.enter_context(tc.tile_pool(name="a_sb", bufs=4))
        w_pool = ctx.enter_context(tc.tile_pool(name="w_sb", bufs=4))
        za_pool = ctx.enter_context(tc.tile_pool(name="za", bufs=5))
        wz_pool = ctx.enter_context(tc.tile_pool(name="wz", bufs=4))
        v_pool = ctx.enter_context(tc.tile_pool(name="vv", bufs=3))
        o_pool = ctx.enter_context(tc.tile_pool(name="o_sb", bufs=4))
        st_pool = ctx.enter_context(tc.tile_pool(name="st", bufs=8))
        rb_pool = ctx.enter_context(tc.tile_pool(name="rb", bufs=2))
        rb16_pool = ctx.enter_context(tc.tile_pool(name="rb16", bufs=3))
        ps_pool = ctx.enter_context(tc.tile_pool(name="ps", bufs=2, space="PSUM"))

        # Slots whose A-side sums come from ACT accum_out (8-way split exp,
        # +~2.0us ACT) instead of the DVE tree+reduce (-~1.7us DVE): engine
        # rebalance, ACT ~88us -> ~105 and DVE ~120 -> ~107.
        ACCUM_SLOTS = frozenset(range(1, NSLOTS, 2))

        def stage_a1(k):
            """Loads of slot k (+ A-side exp for tree slots)."""
            a_sb = a_pool.tile([P, T, B], F16)
            nc.sync.dma_start(a_sb[:], a[k])
            w_sb = w_pool.tile([P, T, B], F16)
            nc.sync.dma_start(w_sb[:], w[k])
            za = za_pool.tile([P, T, B], F16)
            if k not in ACCUM_SLOTS:
                nc.scalar.activation(za[:], a_sb[:], AF.Exp, scale=1.0 / TAU)
            return za, a_sb, w_sb

        def stage_a2(k, za, a_sb, w_sb):
            """W-side exp, PE B-sums, A-sums (tree or ACT accum) of slot k."""
            wz = wz_pool.tile([P, T, B], F16)
            nc.scalar.activation(wz[:], w_sb[:], AF.Exp, scale=1.0 / TAU)

            # B-side group sums on PE: sb_ps[:, g, c] = sum over the 256
            # B-columns of group g (= partitions of wz t-subtiles 2g, 2g+1),
            # broadcast to all 128 output partitions by the all-ones lhsT.
            # The tile carries T spare columns per group that the A-side
            # reduce fills below, so ONE reciprocal_approx_fast covers both
            # normalizations (the approx-recip has ~0.7us fixed cost). Group
            # stride padded to 1024 f32 (2 PSUM banks) so each matmul output
            # sits inside bank boundaries.
            sb_ps = ps_pool.tile([P, NG, 2 * B], F32)
            for t in range(T):
                nc.tensor.matmul(
                    sb_ps[:, t // 2, 0:B], ones[:], wz[:, t, :],
                    start=(t % 2 == 0), stop=(t % 2 == 1),
                )

            sb_flat = sb_ps[:].rearrange("p g c -> p (g c)")
            if k in ACCUM_SLOTS:
                # A-exp as 8 per-group activations whose accum_out drops each
                # group sum straight into the spare PSUM columns.
                for t in range(T):
                    for g in range(NG):
                        cs = slice(g * GRP, (g + 1) * GRP)
                        col = g * (2 * B) + B + t
                        nc.scalar.activation(
                            za[:, t, cs], a_sb[:, t, cs], AF.Exp,
                            scale=1.0 / TAU,
                            accum_out=sb_flat[:, col:col + 1],
                        )
            else:
                # A-side group sums: fp16 tree-add halvings at DVE 2x, then
                # a short f32 tensor_reduce (free-dim reduces are DVE-only,
                # and plain tensor_reduce has no fp16 fast mode).
                za_g = za[:].rearrange("p t b -> p (t b)").rearrange(
                    "p (G s) -> p G s", s=GRP
                )
                s1 = st_pool.tile([P, T * NG, GRP // 2], F16, name="s1")
                nc.vector.tensor_add(
                    s1[:], za_g[:, :, 0:128], za_g[:, :, 128:256])
                s2 = st_pool.tile([P, T * NG, GRP // 4], F16, name="s2")
                nc.vector.tensor_add(s2[:], s1[:, :, 0:64], s1[:, :, 64:128])
                s3 = st_pool.tile([P, T * NG, GRP // 8], F16, name="s3")
                nc.vector.tensor_add(s3[:], s2[:, :, 0:32], s2[:, :, 32:64])
                # A-sums land t-major in the spare columns: [p, g, B + t].
                nc.vector.tensor_reduce(
                    sb_ps[:, :, B:B + T].rearrange("p g t -> p t g"),
                    s3[:], axis=mybir.AxisListType.X, op=OP.add,
                )
            return za, wz, sb_ps

        def stage_b1(k, sb_ps):
            """Combined reciprocal + fp16 downcast of slot k's normalizers.
            Emitted between slot k+1's two exps so the ACT-side downcast runs
            early instead of queueing behind both k+1 activations."""
            rbinv = rb_pool.tile([P, NG, B + T], F32)
            nc.vector.reciprocal_approx_fast(rbinv[:], sb_ps[:, :, 0:B + T])
            rb16 = rb16_pool.tile([P, NG, B + T], F16)
            nc.scalar.activation(rb16[:], rbinv[:], AF.Copy)
            return rb16

        def stage_b2(k, za, wz, rb16):
            """V-multiply of slot k."""
            v = v_pool.tile([P, T, B], F16)
            rb_b = rb16[:, :, 0:B].rearrange("p g (one c) -> p g one c", one=1) \
                .broadcast_to([P, NG, T // NG, B])
            nc.vector.tensor_mul(
                v[:].rearrange("p (g u) b -> p g u b", g=NG), wz[:].rearrange(
                    "p (g u) b -> p g u b", g=NG), rb_b,
            )
            return v

        def stage_b3(k, za, rb16, v):
            """Final product + store of slot k -- emitted one iteration late
            so these always-ready stt fill DVE stalls while the newer slot's
            recip chain crosses PE/ACT."""
            # out = (za * ra) * V: 8 scalar_tensor_tensor; the fp16 scalar
            # 1/sa[t, g] sits at rb16[:, g, B + t].
            o_sb = o_pool.tile([P, T, B], F16)
            for t in range(T):
                for g in range(NG):
                    cs = slice(g * GRP, (g + 1) * GRP)
                    nc.vector.scalar_tensor_tensor(
                        o_sb[:, t, cs], za[:, t, cs],
                        rb16[:, g, B + t:B + t + 1],
                        v[:, t, cs], op0=OP.mult, op1=OP.mult,
                    )
            # Stores ride the SWDGE (gpsimd) ring: they never queue ahead of
            # the sync-ring loads, and the Pool-engine dispatch cost is tiny.
            # Two half-tile stores let the first half leave while the second
            # half's stt work finishes, shortening the per-slot tail.
            nc.gpsimd.dma_start(o[k, :, 0:2], o_sb[:, 0:2, :])
            nc.gpsimd.dma_start(o[k, :, 2:4], o_sb[:, 2:4, :])

        # Software pipelining, one slot of lookahead. Per iteration the
        # emission order is: a1(k+1) [loads + za-exp], b1(k) [recip + rb16
        # downcast -- the ACT copy slips between k+1's exps], a2(k+1)
        # [wz-exp + PE sums + A-tree], b2(k) [V + product + store]. The DVE
        # exec queue then always holds ready tree work while slot k's recip
        # chain crosses PE/ACT, and the rb16 copy isn't stuck behind both
        # k+1 exps on the queue-depth-0 ACT engine.
        # Loads run TWO slots ahead of consumption (exps one ahead): the
        # DMA ring and ACT stream saturate from t=0, cutting the ~20us of
        # pipeline-fill stalls the trace shows in the first few slots.
        a1_cache = {0: stage_a1(0), 1: stage_a1(1)}
        cur = stage_a2(0, *a1_cache.pop(0))
        pend = None
        for k in range(NSLOTS):
            if k + 2 < NSLOTS:
                a1_cache[k + 2] = stage_a1(k + 2)
            nxt = stage_a2(k + 1, *a1_cache.pop(k + 1)) \
                if k + 1 < NSLOTS else None
            rb16 = stage_b1(k, cur[2])
            v = stage_b2(k, cur[0], cur[1], rb16)
            if pend is not None:
                stage_b3(*pend)
            pend = (k, cur[0], rb16, v)
            cur = nxt
        stage_b3(*pend)

    nc.compile()
    return nc


_NC = None


def _get_nc():
    global _NC
    if _NC is None:
        _NC = build()
    return _NC


def _to_pmajor(blocks: np.ndarray) -> np.ndarray:
    # (n, 512, 512) row-major -> (n, 128, 4, 512): row r = t*P + p lands at
    # [p, t, :], so every SBUF partition's bytes are contiguous in DRAM.
    n = blocks.shape[0]
    return np.ascontiguousarray(
        blocks.reshape(n, T, P, B).transpose(0, 2, 1, 3)
    )


def _shifted_fp16(sims: np.ndarray) -> np.ndarray:
    """Y = sims - per-(row, 256-col-group) max, diagonal masked, clipped and
    cast to fp16. Softmax over any 256-aligned column group of Y matches the
    reference's (softmax shift invariance)."""
    y = np.array(sims, dtype=np.float32, copy=True)
    # Mask BEFORE the max: the group max must be over surviving entries,
    # else a dominant diagonal shifts the whole group into fp16 underflow
    # and the on-device group sum becomes 0 (-> inf/NaN).
    np.fill_diagonal(y, -np.inf)
    m = y.reshape(N, N // GRP, GRP).max(axis=-1, keepdims=True)
    y = (y.reshape(N, N // GRP, GRP) - m).reshape(N, N)
    np.clip(y, YCLIP, 0.0, out=y)
    return y.astype(np.float16)


def make_in_maps(sims: np.ndarray) -> list[dict[str, np.ndarray]]:
    yf = _shifted_fp16(sims)
    in_maps = []
    for c in range(NCORES):
        a_stack = np.empty((NSLOTS, B, B), np.float16)
        w_stack = np.empty((NSLOTS, B, B), np.float16)
        for k, (i, j) in enumerate(CORE_PAIRS[c]):
            a_stack[k] = yf[i * B:(i + 1) * B, j * B:(j + 1) * B]
            w_stack[k] = yf[j * B:(j + 1) * B, i * B:(i + 1) * B].T
        in_maps.append({"a": _to_pmajor(a_stack), "w": _to_pmajor(w_stack)})
    return in_maps


def assemble(results: list[dict[str, np.ndarray]]) -> np.ndarray:
    out = np.empty((N, N), np.float32)
    for c in range(NCORES):
        o_pm = results[c]["o"]  # (NSLOTS, P, T, B) fp16 partition-major
        o_stack = o_pm.transpose(0, 2, 1, 3).reshape(NSLOTS, B, B).astype(
            np.float32
        )
        for k, (i, j) in enumerate(CORE_PAIRS[c]):
            out[i * B:(i + 1) * B, j * B:(j + 1) * B] = o_stack[k]
            if i != j:
                out[j * B:(j + 1) * B, i * B:(i + 1) * B] = o_stack[k].T
    return out


def run_on_hw(sims: np.ndarray, **spmd_kwargs):
    """Run the kernel on the 8 NeuronCores. Returns (out, BassKernelResults).

    The device occasionally throws a transient NRT_EXEC_UNIT_UNRECOVERABLE
    and needs ~a minute to come back, so failed runs are retried."""
    import time

    nc = _get_nc()
    in_maps = make_in_maps(sims)
    last_exc = None
    for attempt in range(3):
        if attempt:
            time.sleep(75)
        try:
            res = run_bass_kernel_spmd(
                nc, in_maps, core_ids=list(range(NCORES)), **spmd_kwargs
            )
            return assemble(res.results), res
        except Exception as exc:  # noqa: BLE001 - device flake, retry
            last_exc = exc
    raise last_exc


def kernel(similarities: np.ndarray) -> np.ndarray:
    sims = np.ascontiguousarray(similarities, dtype=np.float32)
    assert sims.shape == (N, N)
    out, _ = run_on_hw(sims)
    return out


if __name__ == "__main__":
    rng = np.random.default_rng(0)
    sims = rng.standard_normal((N, N), dtype=np.float32)
    out = kernel(similarities=sims)
    print("out", out.shape, out.dtype, float(out.max()))
